# revision 25
# baseline (speedup 1.0000x reference)
"""Linear attention (non-causal, elu+1 feature map) on 8 Trainium2 cores.

Math per (batch b, head h), with phi(x) = elu(x)+1 = max(x+1, exp(min(x,0))):
    C_aug = phi(K)^T @ [V | 1]        # (64, 65): context (64x64) + k_sum col
    numer = phi(Q) @ C                # (T, 64)
    denom = phi(Q) @ k_sum            # (T,)
    out   = numer / denom             # eps=1e-6 negligible vs denom ~1e5

Sharding: 16 heads / 8 cores = 2 heads per core, all 4 batches per core.

Everything on device is fp16: halves HBM traffic vs fp32 and the PE runs
fp16 at the same per-column rate as bf16 (measured), while fp16's 10-bit
mantissa keeps worst-element rel err ~1.3e-2 vs the 2e-2 budget (bf16
lands ~9.6e-2). All values fit fp16 range (|C|<=500, ksum<=5000).

Both heads are fused into single 128-wide matmuls:
  mm1: lhsT = [phiK0 | phiK1] (128t x 128), rhs = [VA0 | VA1] (128t x 130)
       -> psum (128 x 130); diagonal 64x65 blocks are C_aug per head
       (cross-head blocks discarded), accumulated over 32 t-tiles.
  mm_d: lhsT = phiQ chunk (128e x 128t), rhs = blockdiag ksum (128 x 2)
       -> denom psum (128t x 2) per chunk, all 32 chunks in one bank so a
       single reciprocal per batch covers them (recip is slow per-call).
  mm2: same lhsT, rhs = blockdiag C (128 x 128) -> numer psum (128t x 128);
       4 chunks fill one 2 KB psum bank exactly, so the normalize+evac is
       one dense 512-col DVE multiply per group (psum fp32 forces 1x mode;
       density and batching are all that's left to win).

phi(x) = elu(x)+1 = min(exp(x), 1) + relu(x); exp runs on ACT straight
off the DMA'd input (x <= 5.5 so exp is finite in fp16; inf would min
away). For K the two parts are never added: mm1 accumulates min-part
and relu-part as two matmuls (C is linear in phiK) — the relu part is
exact in fp16 and the DVE add pass disappears; relu(K) runs on ACT,
which has headroom. For Q (used as mm2/mm_d weights) phi is
materialized with DVE min/relu (4x mode) + add (2x mode).
phi(K) runs in column-chunks chasing the split kv DMA so mm1 starts
~13 us in; phi(Q) overlaps mm1. Engines land balanced: DVE ~50 us,
ACT ~48, DMA ~51-56, PE ~40, wall ~81 us.

Device layouts (per core, all fp16, partition dim first, all APs dense):
    qt: (B, 128, 4096)  qt[b, hh*64+e, n*128+j] = Q[b, t=j*32+n, ch]
    kv: (B, 128, 8256)  cols 0:4096   = K  [n, h, e] (n*128+h*64+e)
                        cols 4096:8256= VA [n, h, m] (n*130+h*65+m, m=64 ones)
                        partition p <-> t = p*32+n
    o:  (B, 128, 4096)  o[b, p, n*128+h*64+e] = out[b, t=p*32+n, h*64+e]
The t = p*32+n tiling gives every DMA 4-8 KB contiguous per partition.
"""

from contextlib import ExitStack

import numpy as np

import concourse.bacc as bacc
import concourse.bass as bass
import concourse.mybir as mybir
import concourse.tile as tile
from concourse.bass_utils import run_bass_kernel_spmd

B = 4
T = 4096
D = 1024
H = 16
E = 64
EA = E + 1
W2 = 2 * EA  # 130 cols: both heads' [VA]
NCORES = 8
HPC = H // NCORES  # 2 heads per core
P = 128
NT = T // P  # 32 t-tiles
KC = HPC * NT * E  # 4096 k-region cols
VC = HPC * NT * EA  # 4160 va-region cols
KV = KC + VC  # 8256
GRP = 8  # mm2 chunks per pso tile (2 psum banks; matmuls stay in-bank)
KSPLIT = 8  # t-tiles in the first (startup) phi-K chunk
DT = mybir.dt.float16
F32 = mybir.dt.float32
AF = mybir.ActivationFunctionType
ALU = mybir.AluOpType
F16 = np.float16


def _phi(nc, x, tmp, act_relu=False):
    """x <- elu(x)+1 == min(exp(x), 1) + relu(x), tmp as scratch.

    exp needs no input clamp: x <= 5.5 so exp(x) <= 245 is finite in
    fp16, and even inf would min() away. ACT leads the chain (starts
    right after the DMA); min/max run in 4x DVE mode, add in 2x.
    act_relu moves the relu to the scalar engine to offload the DVE
    (the overall bottleneck) where ACT has headroom.
    """
    nc.scalar.activation(tmp, x, AF.Exp)
    nc.vector.tensor_scalar_min(tmp, tmp, 1.0)
    if act_relu:
        nc.scalar.activation(x, x, AF.Relu)
    else:
        nc.vector.tensor_scalar_max(x, x, 0.0)
    nc.vector.tensor_tensor(x, x, tmp, ALU.add)


def build_nc():
    nc = bacc.Bacc("TRN2", target_bir_lowering=False, debug=False)
    qt = nc.dram_tensor("qt", [B, P, T], DT, kind="ExternalInput").ap()
    kv = nc.dram_tensor("kv", [B, P, KV], DT, kind="ExternalInput").ap()
    o = nc.dram_tensor("o", [B, P, T], DT, kind="ExternalOutput").ap()

    with tile.TileContext(nc) as tc, ExitStack() as ctx:
        qt_pool = ctx.enter_context(tc.tile_pool(name="qt", bufs=3))
        kv_pool = ctx.enter_context(tc.tile_pool(name="kv", bufs=3))
        tmpk_pool = ctx.enter_context(tc.tile_pool(name="tmpk", bufs=3))
        tmpq_pool = ctx.enter_context(tc.tile_pool(name="tmpq", bufs=2))
        c_pool = ctx.enter_context(tc.tile_pool(name="c", bufs=2))
        ks_pool = ctx.enter_context(tc.tile_pool(name="ks", bufs=2))
        r_pool = ctx.enter_context(tc.tile_pool(name="r", bufs=2))
        out_pool = ctx.enter_context(tc.tile_pool(name="out", bufs=3))
        psc_pool = ctx.enter_context(tc.tile_pool(name="psc", bufs=2, space="PSUM"))
        pso_pool = ctx.enter_context(tc.tile_pool(name="pso", bufs=2, space="PSUM"))
        psd_pool = ctx.enter_context(tc.tile_pool(name="psd", bufs=2, space="PSUM"))

        for b in range(B):
            # interleave small-K / small-VA pieces first so mm1 tiles 0..7
            # have both operands ~4 us before the bulk lands
            kv_t = kv_pool.tile([P, KV], DT)
            c0k = KSPLIT * P
            c0v = KSPLIT * W2
            nc.sync.dma_start(kv_t[:, 0:c0k], kv[b, :, 0:c0k])
            nc.sync.dma_start(kv_t[:, KC : KC + c0v], kv[b, :, KC : KC + c0v])
            nc.sync.dma_start(kv_t[:, c0k:KC], kv[b, :, c0k:KC])
            nc.sync.dma_start(kv_t[:, KC + c0v : KV], kv[b, :, KC + c0v : KV])
            qt_t = qt_pool.tile([P, T], DT)
            nc.sync.dma_start(qt_t[:], qt[b])

            # phi(K) = min(exp K, 1) + relu(K), but the add never happens:
            # mm1 accumulates BOTH parts into psum (C is linear in phiK),
            # killing the K-side DVE add pass. relu(K) is exact in fp16.
            # Per tile: 2 matmuls (relu part from kv_t, min part from tk).
            # First (small) chunk does relu on DVE to skip the serial
            # exp->relu ACT chain on the critical startup path.
            psc = psc_pool.tile([P, 512], F32)  # full psum bank to avoid packing
            for lo_t, hi_t, act_relu in [(0, KSPLIT, False), (KSPLIT, NT, True)]:
                kreg = kv_t[:, lo_t * P : hi_t * P]
                tk = tmpk_pool.tile([P, (hi_t - lo_t) * P], DT)
                nc.scalar.activation(tk[:], kreg, AF.Exp)
                nc.vector.tensor_scalar_min(tk[:], tk[:], 1.0)
                if act_relu:
                    nc.scalar.activation(kreg, kreg, AF.Relu)
                else:
                    nc.vector.tensor_scalar_max(kreg, kreg, 0.0)
                for n in range(lo_t, hi_t):
                    va = kv_t[:, KC + n * W2 : KC + (n + 1) * W2]
                    off = (n - lo_t) * P
                    nc.tensor.matmul(
                        psc[:, 0:W2],
                        lhsT=kv_t[:, n * P : (n + 1) * P],
                        rhs=va,
                        start=(n == 0),
                        stop=False,
                    )
                    nc.tensor.matmul(
                        psc[:, 0:W2],
                        lhsT=tk[:, off : off + P],
                        rhs=va,
                        start=False,
                        stop=(n == NT - 1),
                    )
            # phi(Q): overlaps mm1 on the vector engine
            tq = tmpq_pool.tile([P, T], DT)
            _phi(nc, qt_t[:], tq[:])

            # block-diag C (numer cols only) and ksum, in fp16
            c_sb = c_pool.tile([P, P], DT)
            nc.vector.memset(c_sb[:], 0.0)
            nc.vector.tensor_copy(c_sb[0:E, 0:E], psc[0:E, 0:E])
            nc.vector.tensor_copy(c_sb[E:P, E:P], psc[E:P, EA : EA + E])
            ks_sb = ks_pool.tile([P, HPC], DT)
            nc.vector.memset(ks_sb[:], 0.0)
            nc.vector.tensor_copy(ks_sb[0:E, 0:1], psc[0:E, E : E + 1])
            nc.vector.tensor_copy(ks_sb[E:P, 1:2], psc[E:P, EA + E : W2])

            # denominators for all 32 chunks into one psum bank
            psd = psd_pool.tile([P, 512], F32)  # full psum bank to avoid packing
            for n in range(NT):
                nc.tensor.matmul(
                    psd[:, n * HPC : (n + 1) * HPC],
                    lhsT=qt_t[:, n * P : (n + 1) * P],
                    rhs=ks_sb[:],
                    start=True,
                    stop=True,
                )
            r_sb = r_pool.tile([P, NT * HPC], F32)
            half = NT * HPC // 2
            nc.vector.reciprocal(r_sb[:, 0:half], psd[:, 0:half])
            nc.vector.reciprocal(r_sb[:, half : NT * HPC], psd[:, half : NT * HPC])

            # numerators + normalize, 4 chunks per psum bank; the evac
            # multiply reads psum fp32 so it is 1x mode regardless —
            # batch 512 dense cols per instruction
            out_sb = out_pool.tile([P, T], DT)
            for g in range(NT // GRP):
                pso = pso_pool.tile([P, GRP * P], F32)
                for j in range(GRP):
                    n = g * GRP + j
                    nc.tensor.matmul(
                        pso[:, j * P : (j + 1) * P],
                        lhsT=qt_t[:, n * P : (n + 1) * P],
                        rhs=c_sb[:],
                        start=True,
                        stop=True,
                    )
                ov = out_sb[:, g * GRP * P : (g + 1) * GRP * P].rearrange(
                    "p (n h e) -> p n h e", n=GRP, h=HPC
                )
                iv = pso[:].rearrange("p (n h e) -> p n h e", n=GRP, h=HPC)
                rv = (
                    r_sb[:, g * GRP * HPC : (g + 1) * GRP * HPC]
                    .rearrange("p (n h) -> p n h", h=HPC)
                    .unsqueeze(3)
                    .broadcast_to((P, GRP, HPC, E))
                )
                nc.vector.tensor_tensor(ov, iv, rv, ALU.mult)
                if g == NT // GRP // 2 - 1:
                    nc.sync.dma_start(
                        o[b, :, 0 : T // 2], out_sb[:, 0 : T // 2]
                    )
            nc.sync.dma_start(o[b, :, T // 2 : T], out_sb[:, T // 2 : T])
    nc.finalize()
    return nc


_NC_CACHE = None


def _get_nc():
    global _NC_CACHE
    if _NC_CACHE is None:
        _NC_CACHE = build_nc()
    return _NC_CACHE


def make_in_maps(query, key, value):
    query = np.ascontiguousarray(query, dtype=np.float32)
    key = np.ascontiguousarray(key, dtype=np.float32)
    value = np.ascontiguousarray(value, dtype=np.float32)
    in_maps = []
    for c in range(NCORES):
        lo = c * P
        hi = lo + P
        # qt: col n*128+j <-> t = j*32+n
        qt = query[:, :, lo:hi].transpose(0, 2, 1)  # (B, 128, T) t-major
        qt = np.ascontiguousarray(
            qt.reshape(B, P, P, NT).transpose(0, 1, 3, 2)
        ).reshape(B, P, T)
        # k region: (B, p, n, h, e); t = p*32+n
        kk = key[:, :, lo:hi].reshape(B, P, NT, HPC, E)
        # va region: ones col appended per head, cols (n, h, m)
        va = np.empty((B, P, NT, HPC, EA), np.float32)
        va[..., :E] = value[:, :, lo:hi].reshape(B, P, NT, HPC, E)
        va[..., E] = 1.0
        kvb = np.concatenate(
            [kk.reshape(B, P, KC), va.reshape(B, P, VC)], axis=2
        )
        in_maps.append(
            {"qt": qt.astype(F16), "kv": np.ascontiguousarray(kvb).astype(F16)}
        )
    return in_maps


def assemble_out(results):
    out = np.empty((B, T, D), np.float32)
    for c in range(NCORES):
        oc = np.asarray(results[c]["o"], dtype=np.float32)  # (B, 128, 4096)
        # col = n*128 + h*64 + e; partition p <-> t = p*32+n
        out[:, :, c * P : (c + 1) * P] = oc.reshape(B, T, P)
    return out


def run(query, key, value, **spmd_kwargs):
    nc = _get_nc()
    in_maps = make_in_maps(query, key, value)
    res = run_bass_kernel_spmd(nc, in_maps, core_ids=list(range(NCORES)), **spmd_kwargs)
    return assemble_out(res.results), res


def kernel(query, key, value):
    out, _ = run(query, key, value)
    return out


# revision 26
# speedup vs baseline: 1.0192x; 1.0192x over previous
"""Linear attention (non-causal, elu+1 feature map) on 8 Trainium2 cores.

Math per (batch b, head h), with phi(x) = elu(x)+1 = max(x+1, exp(min(x,0))):
    C_aug = phi(K)^T @ [V | 1]        # (64, 65): context (64x64) + k_sum col
    numer = phi(Q) @ C                # (T, 64)
    denom = phi(Q) @ k_sum            # (T,)
    out   = numer / denom             # eps=1e-6 negligible vs denom ~1e5

Sharding: 16 heads / 8 cores = 2 heads per core, all 4 batches per core.

Everything on device is fp16: halves HBM traffic vs fp32 and the PE runs
fp16 at the same per-column rate as bf16 (measured), while fp16's 10-bit
mantissa keeps worst-element rel err ~1.3e-2 vs the 2e-2 budget (bf16
lands ~9.6e-2). All values fit fp16 range (|C|<=500, ksum<=5000).

Both heads are fused into single 128-wide matmuls:
  mm1: lhsT = [phiK0 | phiK1] (128t x 128), rhs = [VA0 | VA1] (128t x 130)
       -> psum (128 x 130); diagonal 64x65 blocks are C_aug per head
       (cross-head blocks discarded), accumulated over 32 t-tiles.
  mm_d: lhsT = phiQ chunk (128e x 128t), rhs = blockdiag ksum (128 x 2)
       -> denom psum (128t x 2) per chunk, all 32 chunks in one bank so a
       single reciprocal per batch covers them (recip is slow per-call).
  mm2: same lhsT, rhs = blockdiag C (128 x 128) -> numer psum (128t x 128);
       4 chunks fill one 2 KB psum bank exactly, so the normalize+evac is
       one dense 512-col DVE multiply per group (psum fp32 forces 1x mode;
       density and batching are all that's left to win).

phi(x) = elu(x)+1 = min(exp(x), 1) + relu(x); exp runs on ACT straight
off the DMA'd input (x <= 5.5 so exp is finite in fp16; inf would min
away). For K the two parts are never added: mm1 accumulates min-part
and relu-part as two matmuls (C is linear in phiK) — the relu part is
exact in fp16 and the DVE add pass disappears; relu(K) runs on ACT,
which has headroom. For Q (used as mm2/mm_d weights) phi is
materialized with DVE min/relu (4x mode) + add (2x mode).
phi(K) runs in column-chunks chasing the split kv DMA so mm1 starts
~13 us in; phi(Q) overlaps mm1. Engines land balanced: DVE ~50 us,
ACT ~48, DMA ~51-56, PE ~40, wall ~81 us.

Device layouts (per core, all fp16, partition dim first, all APs dense):
    qt: (B, 128, 4096)  qt[b, hh*64+e, n*128+j] = Q[b, t=j*32+n, ch]
    kv: (B, 128, 8256)  cols 0:4096   = K  [n, h, e] (n*128+h*64+e)
                        cols 4096:8256= VA [n, h, m] (n*130+h*65+m, m=64 ones)
                        partition p <-> t = p*32+n
    o:  (B, 128, 4096)  o[b, p, n*128+h*64+e] = out[b, t=p*32+n, h*64+e]
The t = p*32+n tiling gives every DMA 4-8 KB contiguous per partition.
"""

from contextlib import ExitStack

import numpy as np

import concourse.bacc as bacc
import concourse.bass as bass
import concourse.mybir as mybir
import concourse.tile as tile
from concourse.bass_utils import run_bass_kernel_spmd

B = 4
T = 4096
D = 1024
H = 16
E = 64
EA = E + 1
W2 = 2 * EA  # 130 cols: both heads' [VA]
NCORES = 8
HPC = H // NCORES  # 2 heads per core
P = 128
NT = T // P  # 32 t-tiles
KC = HPC * NT * E  # 4096 k-region cols
VC = HPC * NT * EA  # 4160 va-region cols
KV = KC + VC  # 8256
GRP = 4  # mm2 chunks per psum bank (4*128 fp32 = 2048 B = full bank)
KSPLIT = 8  # t-tiles in the first (startup) phi-K chunk
DT = mybir.dt.float16
F32 = mybir.dt.float32
AF = mybir.ActivationFunctionType
ALU = mybir.AluOpType
F16 = np.float16


def _phi(nc, x, tmp, act_relu=False):
    """x <- elu(x)+1 == min(exp(x), 1) + relu(x), tmp as scratch.

    exp needs no input clamp: x <= 5.5 so exp(x) <= 245 is finite in
    fp16, and even inf would min() away. ACT leads the chain (starts
    right after the DMA); min/max run in 4x DVE mode, add in 2x.
    act_relu moves the relu to the scalar engine to offload the DVE
    (the overall bottleneck) where ACT has headroom.
    """
    nc.scalar.activation(tmp, x, AF.Exp)
    nc.vector.tensor_scalar_min(tmp, tmp, 1.0)
    if act_relu:
        nc.scalar.activation(x, x, AF.Relu)
    else:
        nc.vector.tensor_scalar_max(x, x, 0.0)
    nc.vector.tensor_tensor(x, x, tmp, ALU.add)


def build_nc():
    nc = bacc.Bacc("TRN2", target_bir_lowering=False, debug=False)
    qt = nc.dram_tensor("qt", [B, P, T], DT, kind="ExternalInput").ap()
    kv = nc.dram_tensor("kv", [B, P, KV], DT, kind="ExternalInput").ap()
    o = nc.dram_tensor("o", [B, P, T], DT, kind="ExternalOutput").ap()

    with tile.TileContext(nc) as tc, ExitStack() as ctx:
        qt_pool = ctx.enter_context(tc.tile_pool(name="qt", bufs=3))
        kv_pool = ctx.enter_context(tc.tile_pool(name="kv", bufs=3))
        tmpk_pool = ctx.enter_context(tc.tile_pool(name="tmpk", bufs=3))
        tmpq_pool = ctx.enter_context(tc.tile_pool(name="tmpq", bufs=2))
        c_pool = ctx.enter_context(tc.tile_pool(name="c", bufs=2))
        ks_pool = ctx.enter_context(tc.tile_pool(name="ks", bufs=2))
        r_pool = ctx.enter_context(tc.tile_pool(name="r", bufs=2))
        out_pool = ctx.enter_context(tc.tile_pool(name="out", bufs=3))
        psc_pool = ctx.enter_context(tc.tile_pool(name="psc", bufs=2, space="PSUM"))
        pso_pool = ctx.enter_context(tc.tile_pool(name="pso", bufs=4, space="PSUM"))
        psd_pool = ctx.enter_context(tc.tile_pool(name="psd", bufs=2, space="PSUM"))

        for b in range(B):
            # interleave small-K / small-VA pieces first so mm1 tiles 0..7
            # have both operands ~4 us before the bulk lands
            kv_t = kv_pool.tile([P, KV], DT)
            c0k = KSPLIT * P
            c0v = KSPLIT * W2
            nc.sync.dma_start(kv_t[:, 0:c0k], kv[b, :, 0:c0k])
            nc.sync.dma_start(kv_t[:, KC : KC + c0v], kv[b, :, KC : KC + c0v])
            nc.sync.dma_start(kv_t[:, c0k:KC], kv[b, :, c0k:KC])
            nc.sync.dma_start(kv_t[:, KC + c0v : KV], kv[b, :, KC + c0v : KV])
            qt_t = qt_pool.tile([P, T], DT)
            nc.sync.dma_start(qt_t[:], qt[b])

            # phi(K) = min(exp K, 1) + relu(K), but the add never happens:
            # mm1 accumulates BOTH parts into psum (C is linear in phiK),
            # killing the K-side DVE add pass. relu(K) is exact in fp16.
            # Per tile: 2 matmuls (relu part from kv_t, min part from tk).
            # First (small) chunk does relu on DVE to skip the serial
            # exp->relu ACT chain on the critical startup path.
            psc = psc_pool.tile([P, 512], F32)  # full psum bank to avoid packing
            for lo_t, hi_t, act_relu in [(0, KSPLIT, False), (KSPLIT, NT, True)]:
                kreg = kv_t[:, lo_t * P : hi_t * P]
                tk = tmpk_pool.tile([P, (hi_t - lo_t) * P], DT)
                nc.scalar.activation(tk[:], kreg, AF.Exp)
                nc.vector.tensor_scalar_min(tk[:], tk[:], 1.0)
                if act_relu:
                    nc.scalar.activation(kreg, kreg, AF.Relu)
                else:
                    nc.vector.tensor_scalar_max(kreg, kreg, 0.0)
                for n in range(lo_t, hi_t):
                    va = kv_t[:, KC + n * W2 : KC + (n + 1) * W2]
                    off = (n - lo_t) * P
                    nc.tensor.matmul(
                        psc[:, 0:W2],
                        lhsT=kv_t[:, n * P : (n + 1) * P],
                        rhs=va,
                        start=(n == 0),
                        stop=False,
                    )
                    nc.tensor.matmul(
                        psc[:, 0:W2],
                        lhsT=tk[:, off : off + P],
                        rhs=va,
                        start=False,
                        stop=(n == NT - 1),
                    )
            # phi(Q): overlaps mm1 on the vector engine
            tq = tmpq_pool.tile([P, T], DT)
            _phi(nc, qt_t[:], tq[:])

            # block-diag C (numer cols only) and ksum, in fp16
            c_sb = c_pool.tile([P, P], DT)
            nc.vector.memset(c_sb[:], 0.0)
            nc.vector.tensor_copy(c_sb[0:E, 0:E], psc[0:E, 0:E])
            nc.vector.tensor_copy(c_sb[E:P, E:P], psc[E:P, EA : EA + E])
            ks_sb = ks_pool.tile([P, HPC], DT)
            nc.vector.memset(ks_sb[:], 0.0)
            nc.vector.tensor_copy(ks_sb[0:E, 0:1], psc[0:E, E : E + 1])
            nc.vector.tensor_copy(ks_sb[E:P, 1:2], psc[E:P, EA + E : W2])

            # denominators for all 32 chunks into one psum bank
            psd = psd_pool.tile([P, 512], F32)  # full psum bank to avoid packing
            for n in range(NT):
                nc.tensor.matmul(
                    psd[:, n * HPC : (n + 1) * HPC],
                    lhsT=qt_t[:, n * P : (n + 1) * P],
                    rhs=ks_sb[:],
                    start=True,
                    stop=True,
                )
            r_sb = r_pool.tile([P, NT * HPC], F32)
            half = NT * HPC // 2
            nc.vector.reciprocal(r_sb[:, 0:half], psd[:, 0:half])
            nc.vector.reciprocal(r_sb[:, half : NT * HPC], psd[:, half : NT * HPC])

            # numerators + normalize, 4 chunks per psum bank; the evac
            # multiply reads psum fp32 so it is 1x mode regardless —
            # batch 512 dense cols per instruction
            out_sb = out_pool.tile([P, T], DT)
            for g in range(NT // GRP):
                pso = pso_pool.tile([P, GRP * P], F32)
                for j in range(GRP):
                    n = g * GRP + j
                    nc.tensor.matmul(
                        pso[:, j * P : (j + 1) * P],
                        lhsT=qt_t[:, n * P : (n + 1) * P],
                        rhs=c_sb[:],
                        start=True,
                        stop=True,
                    )
                ov = out_sb[:, g * GRP * P : (g + 1) * GRP * P].rearrange(
                    "p (n h e) -> p n h e", n=GRP, h=HPC
                )
                iv = pso[:].rearrange("p (n h e) -> p n h e", n=GRP, h=HPC)
                rv = (
                    r_sb[:, g * GRP * HPC : (g + 1) * GRP * HPC]
                    .rearrange("p (n h) -> p n h", h=HPC)
                    .unsqueeze(3)
                    .broadcast_to((P, GRP, HPC, E))
                )
                nc.vector.tensor_tensor(ov, iv, rv, ALU.mult)
                if g == NT // GRP // 2 - 1:
                    nc.sync.dma_start(
                        o[b, :, 0 : T // 2], out_sb[:, 0 : T // 2]
                    )
            nc.sync.dma_start(o[b, :, T // 2 : T], out_sb[:, T // 2 : T])
    nc.finalize()
    return nc


_NC_CACHE = None


def _get_nc():
    global _NC_CACHE
    if _NC_CACHE is None:
        _NC_CACHE = build_nc()
    return _NC_CACHE


def make_in_maps(query, key, value):
    query = np.ascontiguousarray(query, dtype=np.float32)
    key = np.ascontiguousarray(key, dtype=np.float32)
    value = np.ascontiguousarray(value, dtype=np.float32)
    in_maps = []
    for c in range(NCORES):
        lo = c * P
        hi = lo + P
        # qt: col n*128+j <-> t = j*32+n
        qt = query[:, :, lo:hi].transpose(0, 2, 1)  # (B, 128, T) t-major
        qt = np.ascontiguousarray(
            qt.reshape(B, P, P, NT).transpose(0, 1, 3, 2)
        ).reshape(B, P, T)
        # k region: (B, p, n, h, e); t = p*32+n
        kk = key[:, :, lo:hi].reshape(B, P, NT, HPC, E)
        # va region: ones col appended per head, cols (n, h, m)
        va = np.empty((B, P, NT, HPC, EA), np.float32)
        va[..., :E] = value[:, :, lo:hi].reshape(B, P, NT, HPC, E)
        va[..., E] = 1.0
        kvb = np.concatenate(
            [kk.reshape(B, P, KC), va.reshape(B, P, VC)], axis=2
        )
        in_maps.append(
            {"qt": qt.astype(F16), "kv": np.ascontiguousarray(kvb).astype(F16)}
        )
    return in_maps


def assemble_out(results):
    out = np.empty((B, T, D), np.float32)
    for c in range(NCORES):
        oc = np.asarray(results[c]["o"], dtype=np.float32)  # (B, 128, 4096)
        # col = n*128 + h*64 + e; partition p <-> t = p*32+n
        out[:, :, c * P : (c + 1) * P] = oc.reshape(B, T, P)
    return out


def run(query, key, value, **spmd_kwargs):
    nc = _get_nc()
    in_maps = make_in_maps(query, key, value)
    res = run_bass_kernel_spmd(nc, in_maps, core_ids=list(range(NCORES)), **spmd_kwargs)
    return assemble_out(res.results), res


def kernel(query, key, value):
    out, _ = run(query, key, value)
    return out


# revision 27
# speedup vs baseline: 1.0245x; 1.0052x over previous
"""Linear attention (non-causal, elu+1 feature map) on 8 Trainium2 cores.

Math per (batch b, head h), with phi(x) = elu(x)+1 = max(x+1, exp(min(x,0))):
    C_aug = phi(K)^T @ [V | 1]        # (64, 65): context (64x64) + k_sum col
    numer = phi(Q) @ C                # (T, 64)
    denom = phi(Q) @ k_sum            # (T,)
    out   = numer / denom             # eps=1e-6 negligible vs denom ~1e5

Sharding: 16 heads / 8 cores = 2 heads per core, all 4 batches per core.

Everything on device is fp16: halves HBM traffic vs fp32 and the PE runs
fp16 at the same per-column rate as bf16 (measured), while fp16's 10-bit
mantissa keeps worst-element rel err ~1.3e-2 vs the 2e-2 budget (bf16
lands ~9.6e-2). All values fit fp16 range (|C|<=500, ksum<=5000).

Both heads are fused into single 128-wide matmuls:
  mm1: lhsT = [phiK0 | phiK1] (128t x 128), rhs = [VA0 | VA1] (128t x 130)
       -> psum (128 x 130); diagonal 64x65 blocks are C_aug per head
       (cross-head blocks discarded), accumulated over 32 t-tiles.
  mm_d: lhsT = phiQ chunk (128e x 128t), rhs = blockdiag ksum (128 x 2)
       -> denom psum (128t x 2) per chunk, all 32 chunks in one bank so a
       single reciprocal per batch covers them (recip is slow per-call).
  mm2: same lhsT, rhs = blockdiag C (128 x 128) -> numer psum (128t x 128);
       4 chunks fill one 2 KB psum bank exactly, so the normalize+evac is
       one dense 512-col DVE multiply per group (psum fp32 forces 1x mode;
       density and batching are all that's left to win).

phi(x) = elu(x)+1 = min(exp(x), 1) + relu(x); exp runs on ACT straight
off the DMA'd input (x <= 5.5 so exp is finite in fp16; inf would min
away). For K the two parts are never added: mm1 accumulates min-part
and relu-part as two matmuls (C is linear in phiK) — the relu part is
exact in fp16 and the DVE add pass disappears; relu(K) runs on ACT,
which has headroom. For Q (used as mm2/mm_d weights) phi is
materialized with DVE min/relu (4x mode) + add (2x mode).
phi(K) runs in column-chunks chasing the split kv DMA so mm1 starts
~13 us in; phi(Q) overlaps mm1. Engines land balanced: DVE ~50 us,
ACT ~48, DMA ~51-56, PE ~40, wall ~81 us.

Device layouts (per core, all fp16, partition dim first, all APs dense):
    qt: (B, 128, 4096)  qt[b, hh*64+e, n*128+j] = Q[b, t=j*32+n, ch]
    kv: (B, 128, 8256)  cols 0:4096   = K  [n, h, e] (n*128+h*64+e)
                        cols 4096:8256= VA [n, h, m] (n*130+h*65+m, m=64 ones)
                        partition p <-> t = p*32+n
    o:  (B, 128, 4096)  o[b, p, n*128+h*64+e] = out[b, t=p*32+n, h*64+e]
The t = p*32+n tiling gives every DMA 4-8 KB contiguous per partition.
"""

from contextlib import ExitStack

import numpy as np

import concourse.bacc as bacc
import concourse.bass as bass
import concourse.mybir as mybir
import concourse.tile as tile
from concourse.bass_utils import run_bass_kernel_spmd

B = 4
T = 4096
D = 1024
H = 16
E = 64
EA = E + 1
W2 = 2 * EA  # 130 cols: both heads' [VA]
NCORES = 8
HPC = H // NCORES  # 2 heads per core
P = 128
NT = T // P  # 32 t-tiles
KC = HPC * NT * E  # 4096 k-region cols
VC = HPC * NT * EA  # 4160 va-region cols
KV = KC + VC  # 8256
GRP = 4  # mm2 chunks per psum bank (4*128 fp32 = 2048 B = full bank)
KSPLIT = 8  # t-tiles in the first (startup) phi-K chunk
DT = mybir.dt.float16
F32 = mybir.dt.float32
AF = mybir.ActivationFunctionType
ALU = mybir.AluOpType
F16 = np.float16


def _phi(nc, x, tmp, act_relu=False):
    """x <- elu(x)+1 == min(exp(x), 1) + relu(x), tmp as scratch.

    exp needs no input clamp: x <= 5.5 so exp(x) <= 245 is finite in
    fp16, and even inf would min() away. ACT leads the chain (starts
    right after the DMA); min/max run in 4x DVE mode, add in 2x.
    act_relu moves the relu to the scalar engine to offload the DVE
    (the overall bottleneck) where ACT has headroom.
    """
    nc.scalar.activation(tmp, x, AF.Exp)
    nc.vector.tensor_scalar_min(tmp, tmp, 1.0)
    if act_relu:
        nc.scalar.activation(x, x, AF.Relu)
    else:
        nc.vector.tensor_scalar_max(x, x, 0.0)
    nc.vector.tensor_tensor(x, x, tmp, ALU.add)


def build_nc():
    nc = bacc.Bacc("TRN2", target_bir_lowering=False, debug=False)
    qt = nc.dram_tensor("qt", [B, P, T], DT, kind="ExternalInput").ap()
    kv = nc.dram_tensor("kv", [B, P, KV], DT, kind="ExternalInput").ap()
    o = nc.dram_tensor("o", [B, P, T], DT, kind="ExternalOutput").ap()

    with tile.TileContext(nc) as tc, ExitStack() as ctx:
        qt_pool = ctx.enter_context(tc.tile_pool(name="qt", bufs=3))
        kv_pool = ctx.enter_context(tc.tile_pool(name="kv", bufs=3))
        tmpk_pool = ctx.enter_context(tc.tile_pool(name="tmpk", bufs=3))
        tmpq_pool = ctx.enter_context(tc.tile_pool(name="tmpq", bufs=2))
        c_pool = ctx.enter_context(tc.tile_pool(name="c", bufs=2))
        ks_pool = ctx.enter_context(tc.tile_pool(name="ks", bufs=2))
        r_pool = ctx.enter_context(tc.tile_pool(name="r", bufs=2))
        out_pool = ctx.enter_context(tc.tile_pool(name="out", bufs=3))
        psc_pool = ctx.enter_context(tc.tile_pool(name="psc", bufs=2, space="PSUM"))
        pso_pool = ctx.enter_context(tc.tile_pool(name="pso", bufs=4, space="PSUM"))
        psd_pool = ctx.enter_context(tc.tile_pool(name="psd", bufs=2, space="PSUM"))

        for b in range(B):
            kv_t = kv_pool.tile([P, KV], DT)
            nc.sync.dma_start(kv_t[:, 0 : KC // 2], kv[b, :, 0 : KC // 2])
            nc.sync.dma_start(kv_t[:, KC // 2 : KC], kv[b, :, KC // 2 : KC])
            nc.sync.dma_start(kv_t[:, KC:KV], kv[b, :, KC:KV])
            qt_t = qt_pool.tile([P, T], DT)
            nc.sync.dma_start(qt_t[:], qt[b])

            # phi(K) = min(exp K, 1) + relu(K), but the add never happens:
            # mm1 accumulates BOTH parts into psum (C is linear in phiK),
            # killing the K-side DVE add pass. relu(K) is exact in fp16.
            # Per tile: 2 matmuls (relu part from kv_t, min part from tk).
            psc = psc_pool.tile([P, 512], F32)  # full psum bank to avoid packing
            kchunk = KC // 2
            for c in range(2):
                kreg = kv_t[:, c * kchunk : (c + 1) * kchunk]
                tk = tmpk_pool.tile([P, kchunk], DT)
                nc.scalar.activation(tk[:], kreg, AF.Exp)
                nc.vector.tensor_scalar_min(tk[:], tk[:], 1.0)
                nc.scalar.activation(kreg, kreg, AF.Relu)
                for n in range(c * (NT // 2), (c + 1) * (NT // 2)):
                    va = kv_t[:, KC + n * W2 : KC + (n + 1) * W2]
                    off = (n - c * (NT // 2)) * P
                    nc.tensor.matmul(
                        psc[:, 0:W2],
                        lhsT=kv_t[:, n * P : (n + 1) * P],
                        rhs=va,
                        start=(n == 0),
                        stop=False,
                    )
                    nc.tensor.matmul(
                        psc[:, 0:W2],
                        lhsT=tk[:, off : off + P],
                        rhs=va,
                        start=False,
                        stop=(n == NT - 1),
                    )
            # phi(Q): overlaps mm1 on the vector engine
            tq = tmpq_pool.tile([P, T], DT)
            _phi(nc, qt_t[:], tq[:])

            # block-diag C (numer cols only) and ksum, in fp16
            c_sb = c_pool.tile([P, P], DT)
            nc.vector.memset(c_sb[:], 0.0)
            nc.vector.tensor_copy(c_sb[0:E, 0:E], psc[0:E, 0:E])
            nc.vector.tensor_copy(c_sb[E:P, E:P], psc[E:P, EA : EA + E])
            ks_sb = ks_pool.tile([P, HPC], DT)
            nc.vector.memset(ks_sb[:], 0.0)
            nc.vector.tensor_copy(ks_sb[0:E, 0:1], psc[0:E, E : E + 1])
            nc.vector.tensor_copy(ks_sb[E:P, 1:2], psc[E:P, EA + E : W2])

            # denominators for all 32 chunks into one psum bank
            psd = psd_pool.tile([P, 512], F32)  # full psum bank to avoid packing
            for n in range(NT):
                nc.tensor.matmul(
                    psd[:, n * HPC : (n + 1) * HPC],
                    lhsT=qt_t[:, n * P : (n + 1) * P],
                    rhs=ks_sb[:],
                    start=True,
                    stop=True,
                )
            r_sb = r_pool.tile([P, NT * HPC], F32)
            half = NT * HPC // 2
            nc.vector.reciprocal(r_sb[:, 0:half], psd[:, 0:half])
            nc.vector.reciprocal(r_sb[:, half : NT * HPC], psd[:, half : NT * HPC])

            # numerators + normalize, 4 chunks per psum bank; the evac
            # multiply reads psum fp32 so it is 1x mode regardless —
            # batch 512 dense cols per instruction
            out_sb = out_pool.tile([P, T], DT)
            for g in range(NT // GRP):
                pso = pso_pool.tile([P, GRP * P], F32)
                for j in range(GRP):
                    n = g * GRP + j
                    nc.tensor.matmul(
                        pso[:, j * P : (j + 1) * P],
                        lhsT=qt_t[:, n * P : (n + 1) * P],
                        rhs=c_sb[:],
                        start=True,
                        stop=True,
                    )
                ov = out_sb[:, g * GRP * P : (g + 1) * GRP * P].rearrange(
                    "p (n h e) -> p n h e", n=GRP, h=HPC
                )
                iv = pso[:].rearrange("p (n h e) -> p n h e", n=GRP, h=HPC)
                rv = (
                    r_sb[:, g * GRP * HPC : (g + 1) * GRP * HPC]
                    .rearrange("p (n h) -> p n h", h=HPC)
                    .unsqueeze(3)
                    .broadcast_to((P, GRP, HPC, E))
                )
                nc.vector.tensor_tensor(ov, iv, rv, ALU.mult)
                if g == NT // GRP // 2 - 1:
                    nc.sync.dma_start(
                        o[b, :, 0 : T // 2], out_sb[:, 0 : T // 2]
                    )
            nc.sync.dma_start(o[b, :, T // 2 : T], out_sb[:, T // 2 : T])
    nc.finalize()
    return nc


_NC_CACHE = None


def _get_nc():
    global _NC_CACHE
    if _NC_CACHE is None:
        _NC_CACHE = build_nc()
    return _NC_CACHE


def make_in_maps(query, key, value):
    query = np.ascontiguousarray(query, dtype=np.float32)
    key = np.ascontiguousarray(key, dtype=np.float32)
    value = np.ascontiguousarray(value, dtype=np.float32)
    in_maps = []
    for c in range(NCORES):
        lo = c * P
        hi = lo + P
        # qt: col n*128+j <-> t = j*32+n
        qt = query[:, :, lo:hi].transpose(0, 2, 1)  # (B, 128, T) t-major
        qt = np.ascontiguousarray(
            qt.reshape(B, P, P, NT).transpose(0, 1, 3, 2)
        ).reshape(B, P, T)
        # k region: (B, p, n, h, e); t = p*32+n
        kk = key[:, :, lo:hi].reshape(B, P, NT, HPC, E)
        # va region: ones col appended per head, cols (n, h, m)
        va = np.empty((B, P, NT, HPC, EA), np.float32)
        va[..., :E] = value[:, :, lo:hi].reshape(B, P, NT, HPC, E)
        va[..., E] = 1.0
        kvb = np.concatenate(
            [kk.reshape(B, P, KC), va.reshape(B, P, VC)], axis=2
        )
        in_maps.append(
            {"qt": qt.astype(F16), "kv": np.ascontiguousarray(kvb).astype(F16)}
        )
    return in_maps


def assemble_out(results):
    out = np.empty((B, T, D), np.float32)
    for c in range(NCORES):
        oc = np.asarray(results[c]["o"], dtype=np.float32)  # (B, 128, 4096)
        # col = n*128 + h*64 + e; partition p <-> t = p*32+n
        out[:, :, c * P : (c + 1) * P] = oc.reshape(B, T, P)
    return out


def run(query, key, value, **spmd_kwargs):
    nc = _get_nc()
    in_maps = make_in_maps(query, key, value)
    res = run_bass_kernel_spmd(nc, in_maps, core_ids=list(range(NCORES)), **spmd_kwargs)
    return assemble_out(res.results), res


def kernel(query, key, value):
    out, _ = run(query, key, value)
    return out


# revision 29
# speedup vs baseline: 1.2094x; 1.1804x over previous
"""Linear attention (non-causal, elu+1 feature map) on 8 Trainium2 cores.

Math per (batch b, head h), with phi(x) = elu(x)+1 = max(x+1, exp(min(x,0))):
    C_aug = phi(K)^T @ [V | 1]        # (64, 65): context (64x64) + k_sum col
    numer = phi(Q) @ C                # (T, 64)
    denom = phi(Q) @ k_sum            # (T,)
    out   = numer / denom             # eps=1e-6 negligible vs denom ~1e5

Sharding: 16 heads / 8 cores = 2 heads per core, all 4 batches per core.

Everything on device is fp16: halves HBM traffic vs fp32 and the PE runs
fp16 at the same per-column rate as bf16 (measured), while fp16's 10-bit
mantissa keeps worst-element rel err ~1.3e-2 vs the 2e-2 budget (bf16
lands ~9.6e-2). All values fit fp16 range (|C|<=500, ksum<=5000).

Both heads are fused into single 128-wide matmuls:
  mm1: lhsT = [phiK0 | phiK1] (128t x 128), rhs = [VA0 | VA1] (128t x 130)
       -> psum (128 x 130); diagonal 64x65 blocks are C_aug per head
       (cross-head blocks discarded), accumulated over 32 t-tiles.
  mm_d: lhsT = phiQ chunk (128e x 128t), rhs = blockdiag ksum (128 x 2)
       -> denom psum (128t x 2) per chunk, all 32 chunks in one bank so a
       single reciprocal per batch covers them (recip is slow per-call).
  mm2: same lhsT, rhs = blockdiag C (128 x 128) -> numer psum (128t x 128);
       4 chunks fill one 2 KB psum bank exactly, so the normalize+evac is
       one dense 512-col DVE multiply per group (psum fp32 forces 1x mode;
       density and batching are all that's left to win).

phi(x) = elu(x)+1 = min(exp(x), 1) + relu(x); exp runs on ACT straight
off the DMA'd input (x <= 5.5 so exp is finite in fp16; inf would min
away). For K the two parts are never added: mm1 accumulates min-part
and relu-part as two matmuls (C is linear in phiK) — the relu part is
exact in fp16 and the DVE add pass disappears; relu(K) runs on ACT,
which has headroom. For Q (used as mm2/mm_d weights) phi is
materialized with DVE min/relu (4x mode) + add (2x mode).
phi(K) runs in column-chunks chasing the split kv DMA so mm1 starts
~13 us in; phi(Q) overlaps mm1. Engines land balanced: DVE ~50 us,
ACT ~48, DMA ~51-56, PE ~40, wall ~81 us.

Device layouts (per core, all fp16, partition dim first, all APs dense):
    qt: (B, 128, 4096)  qt[b, hh*64+e, n*128+j] = Q[b, t=j*32+n, ch]
    kv: (B, 128, 8256)  cols 0:4096   = K  [n, h, e] (n*128+h*64+e)
                        cols 4096:8256= VA [n, h, m] (n*130+h*65+m, m=64 ones)
                        partition p <-> t = p*32+n
    o:  (B, 128, 4096)  o[b, p, n*128+h*64+e] = out[b, t=p*32+n, h*64+e]
The t = p*32+n tiling gives every DMA 4-8 KB contiguous per partition.
"""

from contextlib import ExitStack

import numpy as np

import concourse.bacc as bacc
import concourse.bass as bass
import concourse.mybir as mybir
import concourse.tile as tile
from concourse.bass_utils import run_bass_kernel_spmd

B = 4
T = 4096
D = 1024
H = 16
E = 64
EA = E + 1
W2 = 2 * EA  # 130 cols: both heads' [VA]
NCORES = 8
HPC = H // NCORES  # 2 heads per core
P = 128
NT = T // P  # 32 t-tiles
KC = HPC * NT * E  # 4096 k-region cols
VC = HPC * NT * EA  # 4160 va-region cols
KV = KC + VC  # 8256
GRP = 4  # mm2 chunks per psum bank (4*128 fp32 = 2048 B = full bank)
KSPLIT = 8  # t-tiles in the first (startup) phi-K chunk
DT = mybir.dt.float16
F32 = mybir.dt.float32
AF = mybir.ActivationFunctionType
ALU = mybir.AluOpType
F16 = np.float16


def _phi(nc, x, tmp, act_relu=False):
    """x <- elu(x)+1 == min(exp(x), 1) + relu(x), tmp as scratch.

    exp needs no input clamp: x <= 5.5 so exp(x) <= 245 is finite in
    fp16, and even inf would min() away. ACT leads the chain (starts
    right after the DMA); min/max run in 4x DVE mode, add in 2x.
    act_relu moves the relu to the scalar engine to offload the DVE
    (the overall bottleneck) where ACT has headroom.
    """
    nc.scalar.activation(tmp, x, AF.Exp)
    nc.vector.tensor_scalar_min(tmp, tmp, 1.0)
    if act_relu:
        nc.scalar.activation(x, x, AF.Relu)
    else:
        nc.vector.tensor_scalar_max(x, x, 0.0)
    nc.vector.tensor_tensor(x, x, tmp, ALU.add)


def build_nc():
    nc = bacc.Bacc("TRN2", target_bir_lowering=False, debug=False)
    qt = nc.dram_tensor("qt", [B, P, T], DT, kind="ExternalInput").ap()
    kv = nc.dram_tensor("kv", [B, P, KV], DT, kind="ExternalInput").ap()
    o = nc.dram_tensor("o", [B, P, T], DT, kind="ExternalOutput").ap()

    with tile.TileContext(nc) as tc, ExitStack() as ctx:
        qt_pool = ctx.enter_context(tc.tile_pool(name="qt", bufs=3))
        kv_pool = ctx.enter_context(tc.tile_pool(name="kv", bufs=3))
        tmpk_pool = ctx.enter_context(tc.tile_pool(name="tmpk", bufs=3))
        tmpq_pool = ctx.enter_context(tc.tile_pool(name="tmpq", bufs=2))
        c_pool = ctx.enter_context(tc.tile_pool(name="c", bufs=2))
        ks_pool = ctx.enter_context(tc.tile_pool(name="ks", bufs=2))
        r_pool = ctx.enter_context(tc.tile_pool(name="r", bufs=2))
        out_pool = ctx.enter_context(tc.tile_pool(name="out", bufs=3))
        psc_pool = ctx.enter_context(tc.tile_pool(name="psc", bufs=2, space="PSUM"))
        pso_pool = ctx.enter_context(tc.tile_pool(name="pso", bufs=4, space="PSUM"))
        psd_pool = ctx.enter_context(tc.tile_pool(name="psd", bufs=2, space="PSUM"))

        for b in range(B):
            kv_t = kv_pool.tile([P, KV], DT)
            nc.sync.dma_start(kv_t[:, 0 : KC // 2], kv[b, :, 0 : KC // 2])
            nc.sync.dma_start(kv_t[:, KC // 2 : KC], kv[b, :, KC // 2 : KC])
            nc.sync.dma_start(kv_t[:, KC:KV], kv[b, :, KC:KV])
            qt_t = qt_pool.tile([P, T], DT)
            nc.sync.dma_start(qt_t[:], qt[b])

            # phi(K) = min(exp K, 1) + relu(K), but the add never happens:
            # mm1 accumulates BOTH parts into psum (C is linear in phiK),
            # killing the K-side DVE add pass. relu(K) is exact in fp16.
            # Per tile: 2 matmuls (relu part from kv_t, min part from tk).
            psc = psc_pool.tile([P, 512], F32)  # full psum bank to avoid packing
            kchunk = KC // 2
            for c in range(2):
                kreg = kv_t[:, c * kchunk : (c + 1) * kchunk]
                tk = tmpk_pool.tile([P, kchunk], DT)
                nc.scalar.activation(tk[:], kreg, AF.Exp)
                nc.vector.tensor_scalar_min(tk[:], tk[:], 1.0)
                nc.scalar.activation(kreg, kreg, AF.Relu)
                for n in range(c * (NT // 2), (c + 1) * (NT // 2)):
                    va = kv_t[:, KC + n * W2 : KC + (n + 1) * W2]
                    off = (n - c * (NT // 2)) * P
                    nc.tensor.matmul(
                        psc[:, 0:W2],
                        lhsT=kv_t[:, n * P : (n + 1) * P],
                        rhs=va,
                        start=(n == 0),
                        stop=False,
                    )
                    nc.tensor.matmul(
                        psc[:, 0:W2],
                        lhsT=tk[:, off : off + P],
                        rhs=va,
                        start=False,
                        stop=(n == NT - 1),
                    )
            # phi(Q): overlaps mm1 on the vector engine
            tq = tmpq_pool.tile([P, T], DT)
            _phi(nc, qt_t[:], tq[:])

            # block-diag C (numer cols only) and ksum, in fp16. The
            # off-diagonal zeros survive pool-buffer reuse (only the diag
            # blocks are rewritten), so memset just the first 2 batches,
            # and on the otherwise-idle GpSimd engine.
            c_sb = c_pool.tile([P, P], DT)
            ks_sb = ks_pool.tile([P, HPC], DT)
            if b < 2:
                nc.gpsimd.memset(c_sb[:], 0.0)
                nc.gpsimd.memset(ks_sb[:], 0.0)
            nc.vector.tensor_copy(c_sb[0:E, 0:E], psc[0:E, 0:E])
            nc.vector.tensor_copy(c_sb[E:P, E:P], psc[E:P, EA : EA + E])
            nc.vector.tensor_copy(ks_sb[0:E, 0:1], psc[0:E, E : E + 1])
            nc.vector.tensor_copy(ks_sb[E:P, 1:2], psc[E:P, EA + E : W2])

            # denominators for all 32 chunks into one psum bank
            psd = psd_pool.tile([P, 512], F32)  # full psum bank to avoid packing
            for n in range(NT):
                nc.tensor.matmul(
                    psd[:, n * HPC : (n + 1) * HPC],
                    lhsT=qt_t[:, n * P : (n + 1) * P],
                    rhs=ks_sb[:],
                    start=True,
                    stop=True,
                )
            r_sb = r_pool.tile([P, NT * HPC], F32)
            half = NT * HPC // 2
            nc.vector.reciprocal(r_sb[:, 0:half], psd[:, 0:half])
            nc.vector.reciprocal(r_sb[:, half : NT * HPC], psd[:, half : NT * HPC])

            # numerators + normalize, 4 chunks per psum bank; the evac
            # multiply reads psum fp32 so it is 1x mode regardless —
            # batch 512 dense cols per instruction
            out_sb = out_pool.tile([P, T], DT)
            for g in range(NT // GRP):
                pso = pso_pool.tile([P, GRP * P], F32)
                for j in range(GRP):
                    n = g * GRP + j
                    nc.tensor.matmul(
                        pso[:, j * P : (j + 1) * P],
                        lhsT=qt_t[:, n * P : (n + 1) * P],
                        rhs=c_sb[:],
                        start=True,
                        stop=True,
                    )
                ov = out_sb[:, g * GRP * P : (g + 1) * GRP * P].rearrange(
                    "p (n h e) -> p n h e", n=GRP, h=HPC
                )
                iv = pso[:].rearrange("p (n h e) -> p n h e", n=GRP, h=HPC)
                rv = (
                    r_sb[:, g * GRP * HPC : (g + 1) * GRP * HPC]
                    .rearrange("p (n h) -> p n h", h=HPC)
                    .unsqueeze(3)
                    .broadcast_to((P, GRP, HPC, E))
                )
                nc.vector.tensor_tensor(ov, iv, rv, ALU.mult)
                if g == NT // GRP // 2 - 1:
                    nc.sync.dma_start(
                        o[b, :, 0 : T // 2], out_sb[:, 0 : T // 2]
                    )
                elif b == B - 1 and g == 5:
                    # last batch: stream the tail out in quarters so the
                    # final DMA only waits on the last evac group
                    nc.sync.dma_start(
                        o[b, :, T // 2 : 3 * T // 4],
                        out_sb[:, T // 2 : 3 * T // 4],
                    )
            if b == B - 1:
                nc.sync.dma_start(
                    o[b, :, 3 * T // 4 : T], out_sb[:, 3 * T // 4 : T]
                )
            else:
                nc.sync.dma_start(o[b, :, T // 2 : T], out_sb[:, T // 2 : T])
    nc.finalize()
    return nc


_NC_CACHE = None


def _get_nc():
    global _NC_CACHE
    if _NC_CACHE is None:
        _NC_CACHE = build_nc()
    return _NC_CACHE


def make_in_maps(query, key, value):
    query = np.ascontiguousarray(query, dtype=np.float32)
    key = np.ascontiguousarray(key, dtype=np.float32)
    value = np.ascontiguousarray(value, dtype=np.float32)
    in_maps = []
    for c in range(NCORES):
        lo = c * P
        hi = lo + P
        # qt: col n*128+j <-> t = j*32+n
        qt = query[:, :, lo:hi].transpose(0, 2, 1)  # (B, 128, T) t-major
        qt = np.ascontiguousarray(
            qt.reshape(B, P, P, NT).transpose(0, 1, 3, 2)
        ).reshape(B, P, T)
        # k region: (B, p, n, h, e); t = p*32+n
        kk = key[:, :, lo:hi].reshape(B, P, NT, HPC, E)
        # va region: ones col appended per head, cols (n, h, m)
        va = np.empty((B, P, NT, HPC, EA), np.float32)
        va[..., :E] = value[:, :, lo:hi].reshape(B, P, NT, HPC, E)
        va[..., E] = 1.0
        kvb = np.concatenate(
            [kk.reshape(B, P, KC), va.reshape(B, P, VC)], axis=2
        )
        in_maps.append(
            {"qt": qt.astype(F16), "kv": np.ascontiguousarray(kvb).astype(F16)}
        )
    return in_maps


def assemble_out(results):
    out = np.empty((B, T, D), np.float32)
    for c in range(NCORES):
        oc = np.asarray(results[c]["o"], dtype=np.float32)  # (B, 128, 4096)
        # col = n*128 + h*64 + e; partition p <-> t = p*32+n
        out[:, :, c * P : (c + 1) * P] = oc.reshape(B, T, P)
    return out


def run(query, key, value, **spmd_kwargs):
    nc = _get_nc()
    in_maps = make_in_maps(query, key, value)
    res = run_bass_kernel_spmd(nc, in_maps, core_ids=list(range(NCORES)), **spmd_kwargs)
    return assemble_out(res.results), res


def kernel(query, key, value):
    out, _ = run(query, key, value)
    return out
